# revision 1
# baseline (speedup 1.0000x reference)
"""CRC24A encoder (nn_CRCEncoder) as a Bass/Tile kernel on 8 Trainium2 NeuronCores.

Computation (per the reference):
    out = concat([X, (X @ G) mod 2], axis=-1)
with X [16384, 4096] of {0,1} float32 and G [4096, 24] of {0,1} float32.

Strategy: pure data parallel over the batch dim — each of the 8 cores gets a
[2048, 4096] shard and the full (replicated) G. The kernel is HBM-bound
(~64 MiB/core round trip), so everything else hides under the DMA stream:

  - 4 MiB double-tiles (256 rows) ride through SBUF once; loads issue on the
    SP HWDGE ring (nc.sync), stores on the ACT ring (nc.scalar) so the two
    rings run concurrently. Parity bits are written into the staging tile's
    last 24 columns, so each output double-tile leaves in one fully
    contiguous DMA.
  - The parity needs the contraction dim (K) on SBUF partitions: 128x128
    chunks are transposed on the TensorE into a shared PSUM bank (4 chunks
    per bank) and evacuated PSUM->SBUF in one wide copy alternating between
    VectorE and ScalarE.
  - The matmul keeps the 24-column G chunk as the (self-loading) stationary
    operand and streams the transposed X chunk, accumulating the parity
    transposed ([24, 128]) over all 32 K-chunks — an f32 matmul reloads its
    stationary operand every instruction, so a 24-column weight load beats a
    128-column one ~5x (this halved the kernel's PE time).
  - The [24, 128] parity sums transpose back on the TensorE, then mod-2 via
    int32 AND on the VectorE, landing next to X in the staging tile.
"""

import contextlib

import numpy as np

import concourse.mybir as mybir
from concourse import bacc
from concourse.bass_utils import run_bass_kernel_spmd
from concourse.masks import make_identity
from concourse.tile import TileContext

N_CORES = 8
BATCH = 16384
K = 4096
CRC = 24
B_SHARD = BATCH // N_CORES  # 2048 rows per core
P = 128
N_TILES = B_SHARD // P  # 16 row-tiles per core
N_CHUNKS = K // P  # 32 K-chunks
TGROUP = 2  # row-tiles per DMA double-tile
CGROUP = 4  # transposes batched per PSUM bank
FP32 = mybir.dt.float32
I32 = mybir.dt.int32


def _crc_body(
    tc,
    o_d,
    x_d,
    g_d,
    repeats,
    tgroup=TGROUP,
    cgroup=CGROUP,
    x_bufs=4,
    xt_bufs=4,
    pst_bufs=3,
    pp_bufs=3,
    tp_bufs=2,
    copy_mode="alt",  # "alt" | "dve" | "act"
    sw_pipeline=False,
    schedule=None,  # row-tiles per DMA group; tapered ends shorten the
    # single-pass ramp (first load / last store run unaccompanied)
):
    nc = tc.nc
    if schedule is None:
        schedule = [tgroup] * (N_TILES // tgroup)
    assert sum(schedule) == N_TILES
    with contextlib.ExitStack() as stk:
        consts = stk.enter_context(tc.tile_pool(name="consts", bufs=1))
        xpool = stk.enter_context(tc.tile_pool(name="x", bufs=x_bufs))
        xtpool = stk.enter_context(tc.tile_pool(name="xt", bufs=xt_bufs))
        pstpool = stk.enter_context(
            tc.tile_pool(name="pst", bufs=pst_bufs, space="PSUM")
        )
        pppool = stk.enter_context(tc.tile_pool(name="ppar", bufs=pp_bufs, space="PSUM"))
        tppool = stk.enter_context(tc.tile_pool(name="tpar", bufs=tp_bufs, space="PSUM"))
        tpsbpool = stk.enter_context(tc.tile_pool(name="tpsb", bufs=2))
        paripool = stk.enter_context(tc.tile_pool(name="pari", bufs=2))

        ident = consts.tile([P, P], FP32)
        make_identity(nc, ident)
        ident24 = consts.tile([CRC, CRC], FP32)
        make_identity(nc, ident24)
        # G chunk c ([128, 24] rows c*128..(c+1)*128) lives at columns
        # [c*24, (c+1)*24) so each matmul's stationary lhsT is a contiguous
        # 24-column slice (cheap self-loading weight load).
        g_sb = consts.tile([P, N_CHUNKS * CRC], FP32)
        # G rides the store (scalar) ring, which is idle at pass start — on
        # the sync ring it would delay the first X load behind it in FIFO.
        if g_d.shape == [P, N_CHUNKS * CRC]:
            # host-packed chunk-major G: one contiguous 384 KB DMA
            nc.scalar.dma_start(out=g_sb, in_=g_d)
        else:
            # [4096, 24] layout: strided gather (4096 x 96 B descriptors)
            nc.scalar.dma_start(
                out=g_sb.rearrange("p (c m) -> p c m", m=CRC),
                in_=g_d.rearrange("(c p) m -> p c m", p=P),
            )

        if copy_mode == "alt":
            copy_engines = [nc.vector.tensor_copy, nc.scalar.copy]
        elif copy_mode == "dve":
            copy_engines = [nc.vector.tensor_copy]
        else:
            copy_engines = [nc.scalar.copy]

        def one_pass():
            n_copies = 0
            row0 = 0
            for tg in schedule:
                rows = slice(row0 * P, (row0 + tg) * P)
                row0 += tg
                # [128, tg, 4120]: cols 0:4096 hold X, parity lands in
                # 4096:4120, so each output group leaves in one contiguous DMA.
                x2 = xpool.tile([P, tg, K + CRC], FP32, tag="x2")
                nc.sync.dma_start(
                    out=x2[:, :, 0:K],
                    in_=x_d[rows, :].rearrange("(two p) k -> p two k", p=P),
                )
                for two in range(tg):
                    # Parity accumulates transposed: ppT = sum_c G_c.T @ XT_c
                    # = (X @ G).T, shape [24, 128]. G_c is the stationary
                    # operand (24 cols), the transposed X chunk streams.
                    ppT = pppool.tile([CRC, P], FP32)

                    def emit_mms(g, xt):
                        for j in range(cgroup):
                            c = g * cgroup + j
                            nc.tensor.matmul(
                                ppT,
                                g_sb[:, c * CRC : (c + 1) * CRC],
                                xt[:, j],
                                start=(c == 0),
                                stop=(c == N_CHUNKS - 1),
                            )

                    # Software-pipelined: group g's matmuls are emitted after
                    # group g+1's transposes, so the PE keeps transposing
                    # while the PSUM->SBUF copy of group g is in flight.
                    pending = None
                    for g in range(N_CHUNKS // cgroup):
                        pst = pstpool.tile([P, cgroup, P], FP32)
                        for j in range(cgroup):
                            c = g * cgroup + j
                            nc.tensor.transpose(
                                pst[:, j], x2[:, two, c * P : (c + 1) * P], ident
                            )
                        xt = xtpool.tile([P, cgroup, P], FP32)
                        copy_engines[n_copies % len(copy_engines)](xt, pst)
                        n_copies += 1
                        if not sw_pipeline:
                            emit_mms(g, xt)
                            continue
                        if pending is not None:
                            emit_mms(*pending)
                        pending = (g, xt)
                    if sw_pipeline:
                        emit_mms(*pending)
                    # Evacuate [24, 128], transpose back on PE, then mod-2 of
                    # exact-integer f32 sums: cast i32, AND 1, cast back.
                    tpsb = tpsbpool.tile([CRC, P], FP32)
                    nc.vector.tensor_copy(tpsb, ppT)
                    tp = tppool.tile([P, CRC], FP32)
                    nc.tensor.transpose(tp, tpsb, ident24)
                    pari = paripool.tile([P, CRC], I32)
                    nc.vector.tensor_copy(pari, tp)
                    nc.vector.tensor_scalar(
                        pari, pari, 1, None, mybir.AluOpType.bitwise_and
                    )
                    nc.vector.tensor_copy(x2[:, two, K : K + CRC], pari)
                nc.scalar.dma_start(
                    out=o_d[rows, :].rearrange("(two p) k -> p two k", p=P),
                    in_=x2,
                )

        if repeats == 1:
            one_pass()
        else:
            with tc.For_i(0, repeats, 1):
                one_pass()


def pack_g(g_mat: np.ndarray) -> np.ndarray:
    """[4096, 24] -> chunk-major [128, 32*24]: chunk c's rows land in columns
    [c*24, (c+1)*24), row c*128+p on partition p."""
    return np.ascontiguousarray(
        g_mat.reshape(N_CHUNKS, P, CRC).transpose(1, 0, 2).reshape(P, N_CHUNKS * CRC)
    )


TAPER_SCHEDULE = [1, 1, 2, 2, 2, 2, 2, 2, 1, 1]


def build_crc_module(repeats: int = 1):
    nc = bacc.Bacc(
        "TRN2", target_bir_lowering=False, debug=False, num_devices=N_CORES
    )
    x_d = nc.dram_tensor("inputs", [B_SHARD, K], FP32, kind="ExternalInput").ap()
    g_d = nc.dram_tensor(
        "g_packed", [P, N_CHUNKS * CRC], FP32, kind="ExternalInput"
    ).ap()
    o_d = nc.dram_tensor("out", [B_SHARD, K + CRC], FP32, kind="ExternalOutput").ap()
    with TileContext(nc) as tc:
        _crc_body(tc, o_d, x_d, g_d, repeats, schedule=TAPER_SCHEDULE)
    nc.compile()
    return nc


_NC_CACHE = None


def kernel(inputs: np.ndarray, g_mat: np.ndarray) -> np.ndarray:
    global _NC_CACHE
    if _NC_CACHE is None:
        _NC_CACHE = build_crc_module(repeats=1)
    nc = _NC_CACHE

    x = np.ascontiguousarray(np.asarray(inputs, dtype=np.float32))
    g = np.ascontiguousarray(np.asarray(g_mat, dtype=np.float32))
    assert x.shape == (BATCH, K) and g.shape == (K, CRC)
    gp = pack_g(g)

    in_maps = [
        {"inputs": x[i * B_SHARD : (i + 1) * B_SHARD], "g_packed": gp}
        for i in range(N_CORES)
    ]
    res = run_bass_kernel_spmd(nc, in_maps, core_ids=list(range(N_CORES)))
    out = np.concatenate([r["out"] for r in res.results], axis=0)
    return out.astype(np.float32, copy=False)



# revision 3
# speedup vs baseline: 3.0684x; 3.0684x over previous
"""CRC24A encoder (nn_CRCEncoder) as a Bass/Tile kernel on 8 Trainium2 NeuronCores.

Computation (per the reference):
    out = concat([X, (X @ G) mod 2], axis=-1)
with X [16384, 4096] of {0,1} float32 and G [4096, 24] of {0,1} float32.

Strategy: pure data parallel over the batch dim — each of the 8 cores gets a
[2048, 4096] shard and the full (replicated) G. The kernel is HBM-bound, so
the dominant optimization is shrinking the HBM footprint: every value is
exactly 0.0 or 1.0, which fp8 e4m3 represents exactly (0x00 / 0x38), so the
device reads X and writes the full output in 8-bit — 16.8 MiB per core per
pass instead of 67.3 MiB at f32. The host converts f32 <-> byte codes; all
DRAM tensors are declared uint8 so the jax/PJRT transport never sees an fp8
dtype, and device-side APs bitcast to float8e4 where the engines need it.

Per 128-row tile the parity path is:
  - 128x128 fp8 chunks of X transpose on the TensorE (1 cycle/row at fp8 vs
    2 at f32) into PSUM. The ISA requires fp8 transpose output with element
    step 2 and 4B-aligned base, so results land at even bytes of a
    [128, c, 128, 2] tile (odd bytes are don't-care).
  - PSUM->SBUF evacuation goes through an int16 bitcast view so the DVE/ACT
    copy takes the 2-byte fast path; engines alternate between row-tiles.
  - Parity accumulates transposed in fp32 PSUM via DoubleRow fp8 matmuls
    (contraction 256/instruction, 0.5 cycles/row): lhsT is a [128, 2, 32]
    G slice (pair blocks padded 24->32 cols for the mandated 16B-aligned
    pair stride, host-packed to match the transpose's k->partition map),
    rhs the two transposed chunks [128, 2, 128] with an inner step-2 view
    that skips the gap bytes. Exact for sums up to 4096.
  - mod 2 via int32 AND on the VectorE, transpose back on the TensorE, and
    the parity lands as fp8 next to X in the staging tile, so each output
    group leaves in one contiguous DMA (loads ride the SP HWDGE ring, stores
    the ACT ring).
"""

import contextlib

import numpy as np

import concourse.mybir as mybir
from concourse import bacc
from concourse.bass_utils import run_bass_kernel_spmd
from concourse.masks import make_identity
from concourse.tile import TileContext

N_CORES = 8
BATCH = 16384
K = 4096
CRC = 24
GW = 32  # G pair block padded to 32 cols: DoubleRow needs 16B-aligned stride
B_SHARD = BATCH // N_CORES  # 2048 rows per core
P = 128
N_TILES = B_SHARD // P  # 16 row-tiles per core
N_CHUNKS = K // P  # 32 k-chunks of 128
DCH = N_CHUNKS // 2  # 16 double-chunks of 256 k (DoubleRow granularity)
DGROUP = 4  # double-chunks batched per PSUM tile / evacuation copy
FP32 = mybir.dt.float32
FP8 = mybir.dt.float8e4
I16 = mybir.dt.int16
I32 = mybir.dt.int32
U8 = mybir.dt.uint8

FP8_ONE = 0x38  # float8 e4m3 encoding of 1.0

TAPER_SCHEDULE = [1, 1, 2, 2, 2, 2, 2, 2, 1, 1]


def _crc_body(
    tc,
    o_d,
    x_d,
    g_d,
    repeats,
    x_bufs=4,
    xt_bufs=4,
    pst_bufs=3,
    pp_bufs=2,
    tp_bufs=2,
    schedule=None,  # row-tiles per DMA group; tapered ends shorten the
    # single-pass ramp (first load / last store run unaccompanied)
):
    nc = tc.nc
    if schedule is None:
        schedule = list(TAPER_SCHEDULE)
    assert sum(schedule) == N_TILES

    # fp8 views of the byte-typed DRAM tensors
    x_f8 = x_d.bitcast(FP8)
    o_f8 = o_d.bitcast(FP8)
    g_f8 = g_d.bitcast(FP8)

    with contextlib.ExitStack() as stk:
        consts = stk.enter_context(tc.tile_pool(name="consts", bufs=1))
        xpool = stk.enter_context(tc.tile_pool(name="x", bufs=x_bufs))
        xtpool = stk.enter_context(tc.tile_pool(name="xt", bufs=xt_bufs))
        pstpool = stk.enter_context(
            tc.tile_pool(name="pst", bufs=pst_bufs, space="PSUM")
        )
        pppool = stk.enter_context(tc.tile_pool(name="ppar", bufs=pp_bufs, space="PSUM"))
        tppool = stk.enter_context(tc.tile_pool(name="tpar", bufs=tp_bufs, space="PSUM"))
        tpsbpool = stk.enter_context(tc.tile_pool(name="tpsb", bufs=2))
        paripool = stk.enter_context(tc.tile_pool(name="pari", bufs=2))

        ident_f32 = consts.tile([P, P], FP32)
        make_identity(nc, ident_f32)
        ident8 = consts.tile([P, P], FP8)
        nc.vector.tensor_copy(ident8, ident_f32)  # 1.0 is exact in e4m3
        ident24 = consts.tile([CRC, CRC], FP32)
        make_identity(nc, ident24)

        # G host-packed as [128, DCH, 2, 32]: g[p, d, i, m] = G[256d+128i+p, m]
        # (cols 24:32 zero), matching the transpose's k -> partition mapping so
        # each matmul's stationary lhsT is a contiguous DoubleRow slice.
        g_sb = consts.tile([P, DCH, 2, GW], FP8)
        # G rides the store (scalar) ring, which is idle at pass start — on
        # the sync ring it would delay the first X load behind it in FIFO.
        nc.scalar.dma_start(
            out=g_sb,
            in_=g_f8.rearrange("p (d i m) -> p d i m", i=2, m=GW),
        )

        copy_engines = [nc.vector.tensor_copy, nc.scalar.copy]

        def one_pass():
            n_copies = 0
            row0 = 0
            for tg in schedule:
                rows = slice(row0 * P, (row0 + tg) * P)
                row0 += tg
                # [128, tg, 4120] fp8: cols 0:4096 hold X, parity lands in
                # 4096:4120, so each output group leaves in one contiguous DMA.
                x2 = xpool.tile([P, tg, K + CRC], FP8, tag="x2")
                nc.sync.dma_start(
                    out=x2[:, :, 0:K],
                    in_=x_f8[rows, :].rearrange("(g p) k -> p g k", p=P),
                )
                for t in range(tg):
                    # Parity accumulates transposed: ppT = sum_d G_d.T @ XT_d
                    # = (X @ G).T (rows 24:32 zero), via DoubleRow fp8 matmuls.
                    ppT = pppool.tile([GW, P], FP32)
                    for dg in range(DCH // DGROUP):
                        pst = pstpool.tile([P, 2 * DGROUP, P, 2], FP8)
                        for j in range(2 * DGROUP):
                            c = dg * 2 * DGROUP + j  # k-chunk of 128
                            nc.tensor.transpose(
                                pst[:, j, :, 0],
                                x2[:, t, c * P : (c + 1) * P],
                                ident8,
                            )
                        xt = xtpool.tile([P, 2 * DGROUP, P, 2], FP8)
                        # bit-preserving evacuation (junk gap bytes included);
                        # int16 view rides the DVE/ACT 2-byte fast path
                        copy_engines[n_copies % len(copy_engines)](
                            xt.bitcast(I16), pst.bitcast(I16)
                        )
                        n_copies += 1
                        for j in range(DGROUP):
                            d = dg * DGROUP + j
                            nc.tensor.matmul(
                                ppT,
                                g_sb[:, d],
                                xt[:, 2 * j : 2 * j + 2, :, 0],
                                start=(d == 0),
                                stop=(d == DCH - 1),
                                perf_mode=mybir.MatmulPerfMode.DoubleRow,
                            )
                    # Evacuate [24, 128], mod-2 of exact-integer f32 sums
                    # (cast i32, AND 1), transpose back on PE, land as fp8.
                    pari = paripool.tile([CRC, P], I32)
                    nc.vector.tensor_copy(pari, ppT[0:CRC])
                    nc.vector.tensor_scalar(
                        pari, pari, 1, None, mybir.AluOpType.bitwise_and
                    )
                    tpsb = tpsbpool.tile([CRC, P], FP32)
                    nc.vector.tensor_copy(tpsb, pari)
                    tp = tppool.tile([P, CRC], FP32)
                    nc.tensor.transpose(tp, tpsb, ident24)
                    nc.vector.tensor_copy(x2[:, t, K : K + CRC], tp)
                nc.scalar.dma_start(
                    out=o_f8[rows, :].rearrange("(g p) k -> p g k", p=P),
                    in_=x2,
                )

        if repeats == 1:
            one_pass()
        else:
            with tc.For_i(0, repeats, 1):
                one_pass()


def pack_g(g_mat: np.ndarray) -> np.ndarray:
    """[4096, 24] {0,1} -> fp8-coded uint8 [128, DCH*2*32] with
    g[p, (d, i), 0:24] = G[256d+128i+p, :] and cols 24:32 zero."""
    g = np.asarray(g_mat)
    gp = np.zeros((P, DCH, 2, GW), np.uint8)
    gp[:, :, :, :CRC] = (
        g.reshape(DCH, 2, P, CRC).transpose(2, 0, 1, 3) != 0
    ).astype(np.uint8) * FP8_ONE
    return np.ascontiguousarray(gp.reshape(P, DCH * 2 * GW))


def encode_x(x: np.ndarray) -> np.ndarray:
    """{0,1} float32 -> fp8 e4m3 byte codes {0x00, 0x38} as uint8."""
    return np.ascontiguousarray(x.astype(np.uint8) * FP8_ONE)


def decode_out(out_u8: np.ndarray) -> np.ndarray:
    """fp8 e4m3 byte codes back to {0,1} float32."""
    return (out_u8 != 0).astype(np.float32)


def build_crc_module(repeats: int = 1):
    nc = bacc.Bacc(
        "TRN2", target_bir_lowering=False, debug=False, num_devices=N_CORES
    )
    x_d = nc.dram_tensor("inputs", [B_SHARD, K], U8, kind="ExternalInput").ap()
    g_d = nc.dram_tensor(
        "g_packed", [P, DCH * 2 * GW], U8, kind="ExternalInput"
    ).ap()
    o_d = nc.dram_tensor("out", [B_SHARD, K + CRC], U8, kind="ExternalOutput").ap()
    with TileContext(nc) as tc:
        _crc_body(tc, o_d, x_d, g_d, repeats)
    nc.compile()
    return nc


_NC_CACHE = None


def kernel(inputs: np.ndarray, g_mat: np.ndarray) -> np.ndarray:
    global _NC_CACHE
    if _NC_CACHE is None:
        _NC_CACHE = build_crc_module(repeats=1)
    nc = _NC_CACHE

    x = np.asarray(inputs, dtype=np.float32)
    g = np.asarray(g_mat, dtype=np.float32)
    assert x.shape == (BATCH, K) and g.shape == (K, CRC)
    x8 = encode_x(x)
    gp = pack_g(g)

    in_maps = [
        {"inputs": x8[i * B_SHARD : (i + 1) * B_SHARD], "g_packed": gp}
        for i in range(N_CORES)
    ]
    res = run_bass_kernel_spmd(nc, in_maps, core_ids=list(range(N_CORES)))
    out = np.concatenate([decode_out(r["out"]) for r in res.results], axis=0)
    return out


# revision 4
# speedup vs baseline: 4.3202x; 1.4079x over previous
"""CRC24A encoder (nn_CRCEncoder) as a Bass/Tile kernel on 8 Trainium2 NeuronCores.

Computation (per the reference):
    out = concat([X, (X @ G) mod 2], axis=-1)
with X [16384, 4096] of {0,1} float32 and G [4096, 24] of {0,1} float32.

Strategy: pure data parallel over the batch dim — each of the 8 cores gets a
[2048, 4096] shard and the full (replicated) G. Two layout decisions make the
device side a pure streaming kernel at the HBM roofline:

  - 8-bit I/O. Every value is exactly 0.0 or 1.0, which fp8 e4m3 represents
    exactly (0x00 / 0x38), so the device moves 16.8 MiB per core per pass
    instead of 67.3 MiB at f32. The host converts f32 <-> byte codes; DRAM
    tensors are declared uint8 so jax/PJRT never sees an fp8 dtype, and
    device-side APs bitcast to float8e4 where the engines need it.
  - k-major (transposed) layout. The host uploads X.T [4096, 2048] and reads
    back the output k-major [4120, 2048]. Loads then put the contraction dim
    on SBUF partitions directly: no TensorE transposes, no PSUM evacuation
    copies — the parity matmul streams straight from the DMA staging tile,
    and the X passthrough stores bit-verbatim from the same tile. The parity
    (X @ G).T [24, rows] is itself k-major, landing as output rows 4096:4119
    with no transpose-back. Host-side de-transposition happens once per
    kernel() call.

Per 512-row block: 16 DoubleRow fp8 matmuls (contraction 256/instruction,
0.5 cycles/row, moving free dim 512 — the full-win regime) accumulate
(X @ G).T in fp32 PSUM, exact for sums up to 4096; the G pair blocks are
host-padded 24->32 cols for the mandated 16B-aligned pair stride. mod 2 is
an int32 AND on the VectorE. Loads ride the SP HWDGE ring, stores the ACT
ring, so the two rings stream concurrently.
"""

import contextlib

import numpy as np

import concourse.mybir as mybir
from concourse import bacc
from concourse.bass_utils import run_bass_kernel_spmd
from concourse.tile import TileContext

N_CORES = 8
BATCH = 16384
K = 4096
CRC = 24
GW = 32  # G pair block padded to 32 cols: DoubleRow needs 16B-aligned stride
B_SHARD = BATCH // N_CORES  # 2048 rows per core
P = 128
N_CHUNKS = K // P  # 32 k-chunks of 128
DCH = N_CHUNKS // 2  # 16 double-chunks of 256 k (DoubleRow granularity)
RB = 512  # rows per block: DMA descriptors >= 512B, PSUM bank exactly filled
N_RB = B_SHARD // RB  # 4 row-blocks per core
FP32 = mybir.dt.float32
FP8 = mybir.dt.float8e4
I32 = mybir.dt.int32
U8 = mybir.dt.uint8

FP8_ONE = 0x38  # float8 e4m3 encoding of 1.0


def _crc_body(
    tc,
    o_d,  # [K + CRC, B_SHARD] uint8, k-major output
    x_d,  # [K, B_SHARD] uint8, k-major input (X.T byte codes)
    g_d,  # [P, DCH * 2 * GW] uint8 packed G
    repeats,
    x_bufs=4,
    pp_bufs=3,
):
    nc = tc.nc
    x_f8 = x_d.bitcast(FP8)
    o_f8 = o_d.bitcast(FP8)
    g_f8 = g_d.bitcast(FP8)

    with contextlib.ExitStack() as stk:
        consts = stk.enter_context(tc.tile_pool(name="consts", bufs=1))
        xpool = stk.enter_context(tc.tile_pool(name="x", bufs=x_bufs))
        pppool = stk.enter_context(tc.tile_pool(name="pp", bufs=pp_bufs, space="PSUM"))
        paripool = stk.enter_context(tc.tile_pool(name="pari", bufs=2))
        parspool = stk.enter_context(tc.tile_pool(name="pars", bufs=2))

        # G host-packed as [128, DCH, 2, 32]: g[p, d, i, m] = G[256d+128i+p, m]
        # (cols 24:32 zero) so each matmul's stationary lhsT is a contiguous
        # DoubleRow slice with a 32B pair stride.
        g_sb = consts.tile([P, DCH, 2, GW], FP8)
        # G rides the store (scalar) ring, which is idle at pass start — on
        # the sync ring it would delay the first X load behind it in FIFO.
        nc.scalar.dma_start(
            out=g_sb,
            in_=g_f8.rearrange("p (d i m) -> p d i m", i=2, m=GW),
        )

        def one_pass():
            # parity for the whole pass, k-major [24, 2048]; stored once
            pars = parspool.tile([CRC, B_SHARD], FP8, tag="pars")
            for b in range(N_RB):
                cols = slice(b * RB, (b + 1) * RB)
                # [128 k, 32 chunks, RB rows] fp8 — X.T block rides through
                # SBUF once: matmul rhs and store source alike.
                x2 = xpool.tile([P, N_CHUNKS, RB], FP8, tag="x2")
                nc.sync.dma_start(
                    out=x2,
                    in_=x_f8[:, cols].rearrange("(c p) r -> p c r", p=P),
                )
                # ppT = sum_d G_d.T @ XT_d = (X @ G).T (rows 24:32 zero)
                ppT = pppool.tile([GW, RB], FP32)
                for d in range(DCH):
                    nc.tensor.matmul(
                        ppT,
                        g_sb[:, d],
                        x2[:, 2 * d : 2 * d + 2],
                        start=(d == 0),
                        stop=(d == DCH - 1),
                        perf_mode=mybir.MatmulPerfMode.DoubleRow,
                    )
                # mod-2 of exact-integer f32 sums: cast i32, AND 1, to fp8
                pari = paripool.tile([CRC, RB], I32)
                nc.vector.tensor_copy(pari, ppT[0:CRC])
                nc.vector.tensor_scalar(
                    pari, pari, 1, None, mybir.AluOpType.bitwise_and
                )
                nc.vector.tensor_copy(pars[:, cols], pari)
                nc.scalar.dma_start(
                    out=o_f8[0:K, cols].rearrange("(c p) r -> p c r", p=P),
                    in_=x2,
                )
            nc.scalar.dma_start(out=o_f8[K : K + CRC, :], in_=pars)

        if repeats == 1:
            one_pass()
        else:
            with tc.For_i(0, repeats, 1):
                one_pass()


def pack_g(g_mat: np.ndarray) -> np.ndarray:
    """[4096, 24] {0,1} -> fp8-coded uint8 [128, DCH*2*32] with
    g[p, (d, i), 0:24] = G[256d+128i+p, :] and cols 24:32 zero."""
    g = np.asarray(g_mat)
    gp = np.zeros((P, DCH, 2, GW), np.uint8)
    gp[:, :, :, :CRC] = (
        g.reshape(DCH, 2, P, CRC).transpose(2, 0, 1, 3) != 0
    ).astype(np.uint8) * FP8_ONE
    return np.ascontiguousarray(gp.reshape(P, DCH * 2 * GW))


def encode_xt(x_shard: np.ndarray) -> np.ndarray:
    """{0,1} float32 [rows, K] -> k-major fp8 byte codes [K, rows] uint8."""
    return np.ascontiguousarray(x_shard.astype(np.uint8).T * FP8_ONE)


def decode_out_t(out_u8: np.ndarray) -> np.ndarray:
    """k-major fp8 byte codes [K+CRC, rows] back to {0,1} f32 [rows, K+CRC]."""
    return (out_u8.T != 0).astype(np.float32)


def build_crc_module(repeats: int = 1):
    nc = bacc.Bacc(
        "TRN2", target_bir_lowering=False, debug=False, num_devices=N_CORES
    )
    x_d = nc.dram_tensor("inputs_t", [K, B_SHARD], U8, kind="ExternalInput").ap()
    g_d = nc.dram_tensor(
        "g_packed", [P, DCH * 2 * GW], U8, kind="ExternalInput"
    ).ap()
    o_d = nc.dram_tensor(
        "out_t", [K + CRC, B_SHARD], U8, kind="ExternalOutput"
    ).ap()
    with TileContext(nc) as tc:
        _crc_body(tc, o_d, x_d, g_d, repeats)
    nc.compile()
    return nc


_NC_CACHE = None


def kernel(inputs: np.ndarray, g_mat: np.ndarray) -> np.ndarray:
    global _NC_CACHE
    if _NC_CACHE is None:
        _NC_CACHE = build_crc_module(repeats=1)
    nc = _NC_CACHE

    x = np.asarray(inputs, dtype=np.float32)
    g = np.asarray(g_mat, dtype=np.float32)
    assert x.shape == (BATCH, K) and g.shape == (K, CRC)
    gp = pack_g(g)

    in_maps = [
        {
            "inputs_t": encode_xt(x[i * B_SHARD : (i + 1) * B_SHARD]),
            "g_packed": gp,
        }
        for i in range(N_CORES)
    ]
    res = run_bass_kernel_spmd(nc, in_maps, core_ids=list(range(N_CORES)))
    out = np.concatenate(
        [decode_out_t(r["out_t"]) for r in res.results], axis=0
    )
    return out


# revision 7
# speedup vs baseline: 4.5501x; 1.0532x over previous
"""CRC24A encoder (nn_CRCEncoder) as a Bass/Tile kernel on 8 Trainium2 NeuronCores.

Computation (per the reference):
    out = concat([X, (X @ G) mod 2], axis=-1)
with X [16384, 4096] of {0,1} float32 and G [4096, 24] of {0,1} float32.

Strategy: pure data parallel over the batch dim — each of the 8 cores gets a
[2048, 4096] shard and the full (replicated) G. Two layout decisions make the
device side a pure streaming kernel at the HBM roofline:

  - 8-bit I/O. Every value is exactly 0.0 or 1.0, which fp8 e4m3 represents
    exactly (0x00 / 0x38), so the device moves 16.8 MiB per core per pass
    instead of 67.3 MiB at f32. The host converts f32 <-> byte codes; DRAM
    tensors are declared uint8 so jax/PJRT never sees an fp8 dtype, and
    device-side APs bitcast to float8e4 where the engines need it.
  - k-major (transposed) layout. The host uploads X.T [4096, 2048] and reads
    back the output k-major [4120, 2048]. Loads then put the contraction dim
    on SBUF partitions directly: no TensorE transposes, no PSUM evacuation
    copies — the parity matmul streams straight from the DMA staging tile,
    and the X passthrough stores bit-verbatim from the same tile. The parity
    (X @ G).T [24, rows] is itself k-major, landing as output rows 4096:4119
    with no transpose-back. Host-side de-transposition happens once per
    kernel() call.

Per 512-row block: 16 DoubleRow fp8 matmuls (contraction 256/instruction,
0.5 cycles/row, moving free dim 512 — the full-win regime) accumulate
(X @ G).T in fp32 PSUM, exact for sums up to 4096; the G pair blocks are
host-padded 24->32 cols for the mandated 16B-aligned pair stride. mod 2 is
an int32 AND on the VectorE. Loads ride the SP HWDGE ring, stores the ACT
ring, so the two rings stream concurrently.
"""

import contextlib

import numpy as np

import concourse.mybir as mybir
from concourse import bacc
from concourse.bass_utils import run_bass_kernel_spmd
from concourse.tile import TileContext

N_CORES = 8
BATCH = 16384
K = 4096
CRC = 24
GW = 32  # G pair block padded to 32 cols: DoubleRow needs 16B-aligned stride
B_SHARD = BATCH // N_CORES  # 2048 rows per core
P = 128
N_CHUNKS = K // P  # 32 k-chunks of 128
DCH = N_CHUNKS // 2  # 16 double-chunks of 256 k (DoubleRow granularity)
RB = 1024  # rows per block: 1 KiB DMA descriptors, well over line-rate minimum
N_RB = B_SHARD // RB  # row-blocks per core
MB = 512  # rows per matmul: fp8 moving operand max is 1024 (= 2*MB), and
# each [32, 512] fp32 accumulator exactly fills one PSUM bank
N_MB = RB // MB
FP32 = mybir.dt.float32
FP8 = mybir.dt.float8e4
I32 = mybir.dt.int32
U8 = mybir.dt.uint8

FP8_ONE = 0x38  # float8 e4m3 encoding of 1.0


def _crc_body(
    tc,
    o_d,  # [K + CRC, B_SHARD] uint8, k-major output
    x_d,  # [K, B_SHARD] uint8, k-major input (X.T byte codes)
    g_d,  # [P, DCH * 2 * GW] uint8 packed G
    repeats,
    x_bufs=3,
    pp_bufs=3,
):
    nc = tc.nc
    x_f8 = x_d.bitcast(FP8)
    o_f8 = o_d.bitcast(FP8)
    g_f8 = g_d.bitcast(FP8)

    with contextlib.ExitStack() as stk:
        consts = stk.enter_context(tc.tile_pool(name="consts", bufs=1))
        xpool = stk.enter_context(tc.tile_pool(name="x", bufs=x_bufs))
        pppool = stk.enter_context(tc.tile_pool(name="pp", bufs=pp_bufs, space="PSUM"))
        paripool = stk.enter_context(tc.tile_pool(name="pari", bufs=2))
        parspool = stk.enter_context(tc.tile_pool(name="pars", bufs=2))

        # G host-packed as [128, DCH, 2, 32]: g[p, d, i, m] = G[256d+128i+p, m]
        # (cols 24:32 zero) so each matmul's stationary lhsT is a contiguous
        # DoubleRow slice with a 32B pair stride.
        g_sb = consts.tile([P, DCH, 2, GW], FP8)
        # G rides the store (scalar) ring, which is idle at pass start — on
        # the sync ring it would delay the first X load behind it in FIFO.
        nc.scalar.dma_start(
            out=g_sb,
            in_=g_f8.rearrange("p (d i m) -> p d i m", i=2, m=GW),
        )

        def one_pass():
            # parity for the whole pass, k-major [24, 2048]; stored once
            pars = parspool.tile([CRC, B_SHARD], FP8, tag="pars")
            for b in range(N_RB):
                cols = slice(b * RB, (b + 1) * RB)
                # [128 k, 32 chunks, RB rows] fp8 — X.T block rides through
                # SBUF once: matmul rhs and store source alike.
                x2 = xpool.tile([P, N_CHUNKS, RB], FP8, tag="x2")
                nc.sync.dma_start(
                    out=x2,
                    in_=x_f8[:, cols].rearrange("(c p) r -> p c r", p=P),
                )
                # ppT = sum_d G_d.T @ XT_d = (X @ G).T (rows 24:32 zero);
                # one accumulation group per 512-row sub-slice / PSUM bank
                ppT = pppool.tile([GW, RB], FP32)
                for d in range(DCH):
                    for s in range(N_MB):
                        nc.tensor.matmul(
                            ppT[:, s * MB : (s + 1) * MB],
                            g_sb[:, d],
                            x2[:, 2 * d : 2 * d + 2, s * MB : (s + 1) * MB],
                            start=(d == 0),
                            stop=(d == DCH - 1),
                            perf_mode=mybir.MatmulPerfMode.DoubleRow,
                        )
                # mod-2 of exact-integer f32 sums: cast i32, AND 1, to fp8
                pari = paripool.tile([CRC, RB], I32)
                nc.vector.tensor_copy(pari, ppT[0:CRC])
                nc.vector.tensor_scalar(
                    pari, pari, 1, None, mybir.AluOpType.bitwise_and
                )
                nc.vector.tensor_copy(pars[:, cols], pari)
                nc.scalar.dma_start(
                    out=o_f8[0:K, cols].rearrange("(c p) r -> p c r", p=P),
                    in_=x2,
                )
            nc.scalar.dma_start(out=o_f8[K : K + CRC, :], in_=pars)

        if repeats == 1:
            one_pass()
        else:
            with tc.For_i(0, repeats, 1):
                one_pass()


def pack_g(g_mat: np.ndarray) -> np.ndarray:
    """[4096, 24] {0,1} -> fp8-coded uint8 [128, DCH*2*32] with
    g[p, (d, i), 0:24] = G[256d+128i+p, :] and cols 24:32 zero."""
    g = np.asarray(g_mat)
    gp = np.zeros((P, DCH, 2, GW), np.uint8)
    gp[:, :, :, :CRC] = (
        g.reshape(DCH, 2, P, CRC).transpose(2, 0, 1, 3) != 0
    ).astype(np.uint8) * FP8_ONE
    return np.ascontiguousarray(gp.reshape(P, DCH * 2 * GW))


def encode_xt(x_shard: np.ndarray) -> np.ndarray:
    """{0,1} float32 [rows, K] -> k-major fp8 byte codes [K, rows] uint8."""
    return np.ascontiguousarray(x_shard.astype(np.uint8).T * FP8_ONE)


def decode_out_t(out_u8: np.ndarray) -> np.ndarray:
    """k-major fp8 byte codes [K+CRC, rows] back to {0,1} f32 [rows, K+CRC]."""
    return (out_u8.T != 0).astype(np.float32)


def build_crc_module(repeats: int = 1):
    nc = bacc.Bacc(
        "TRN2", target_bir_lowering=False, debug=False, num_devices=N_CORES
    )
    x_d = nc.dram_tensor("inputs_t", [K, B_SHARD], U8, kind="ExternalInput").ap()
    g_d = nc.dram_tensor(
        "g_packed", [P, DCH * 2 * GW], U8, kind="ExternalInput"
    ).ap()
    o_d = nc.dram_tensor(
        "out_t", [K + CRC, B_SHARD], U8, kind="ExternalOutput"
    ).ap()
    with TileContext(nc) as tc:
        _crc_body(tc, o_d, x_d, g_d, repeats)
    nc.compile()
    return nc


_NC_CACHE = None


def kernel(inputs: np.ndarray, g_mat: np.ndarray) -> np.ndarray:
    global _NC_CACHE
    if _NC_CACHE is None:
        _NC_CACHE = build_crc_module(repeats=1)
    nc = _NC_CACHE

    x = np.asarray(inputs, dtype=np.float32)
    g = np.asarray(g_mat, dtype=np.float32)
    assert x.shape == (BATCH, K) and g.shape == (K, CRC)
    gp = pack_g(g)

    in_maps = [
        {
            "inputs_t": encode_xt(x[i * B_SHARD : (i + 1) * B_SHARD]),
            "g_packed": gp,
        }
        for i in range(N_CORES)
    ]
    res = run_bass_kernel_spmd(nc, in_maps, core_ids=list(range(N_CORES)))
    out = np.concatenate(
        [decode_out_t(r["out_t"]) for r in res.results], axis=0
    )
    return out


# revision 8
# speedup vs baseline: 4.6655x; 1.0254x over previous
"""CRC24A encoder (nn_CRCEncoder) as a Bass/Tile kernel on 8 Trainium2 NeuronCores.

Computation (per the reference):
    out = concat([X, (X @ G) mod 2], axis=-1)
with X [16384, 4096] of {0,1} float32 and G [4096, 24] of {0,1} float32.

Strategy: pure data parallel over the batch dim — each of the 8 cores gets a
[2048, 4096] shard and the full (replicated) G. Two layout decisions make the
device side a pure streaming kernel at the HBM roofline:

  - 8-bit I/O. Every value is exactly 0.0 or 1.0, which fp8 e4m3 represents
    exactly (0x00 / 0x38), so the device moves 16.8 MiB per core per pass
    instead of 67.3 MiB at f32. The host converts f32 <-> byte codes; DRAM
    tensors are declared uint8 so jax/PJRT never sees an fp8 dtype, and
    device-side APs bitcast to float8e4 where the engines need it.
  - k-major (transposed) layout. The host uploads X.T [4096, 2048] and reads
    back the output k-major [4120, 2048]. Loads then put the contraction dim
    on SBUF partitions directly: no TensorE transposes, no PSUM evacuation
    copies — the parity matmul streams straight from the DMA staging tile,
    and the X passthrough stores bit-verbatim from the same tile. The parity
    (X @ G).T [24, rows] is itself k-major, landing as output rows 4096:4119
    with no transpose-back. Host-side de-transposition happens once per
    kernel() call.

Per 512-row block: 16 DoubleRow fp8 matmuls (contraction 256/instruction,
0.5 cycles/row, moving free dim 512 — the full-win regime) accumulate
(X @ G).T in fp32 PSUM, exact for sums up to 4096; the G pair blocks are
host-padded 24->32 cols for the mandated 16B-aligned pair stride. mod 2 is
an int32 AND on the VectorE. Loads ride the SP HWDGE ring, stores the ACT
ring, so the two rings stream concurrently.
"""

import contextlib

import numpy as np

import concourse.mybir as mybir
from concourse import bacc
from concourse.bass_utils import run_bass_kernel_spmd
from concourse.tile import TileContext

N_CORES = 8
BATCH = 16384
K = 4096
CRC = 24
GW = 32  # G pair block padded to 32 cols: DoubleRow needs 16B-aligned stride
B_SHARD = BATCH // N_CORES  # 2048 rows per core
P = 128
N_CHUNKS = K // P  # 32 k-chunks of 128
DCH = N_CHUNKS // 2  # 16 double-chunks of 256 k (DoubleRow granularity)
RB = 1024  # rows per block: 1 KiB DMA descriptors, well over line-rate minimum
N_RB = B_SHARD // RB  # row-blocks per core
MB = 512  # rows per matmul: fp8 moving operand max is 1024 (= 2*MB), and
# each [32, 512] fp32 accumulator exactly fills one PSUM bank
N_MB = RB // MB
FP32 = mybir.dt.float32
FP8 = mybir.dt.float8e4
I32 = mybir.dt.int32
U8 = mybir.dt.uint8

FP8_ONE = 0x38  # float8 e4m3 encoding of 1.0


def _crc_body(
    tc,
    o_d,  # [K + CRC, B_SHARD] uint8, k-major output
    x_d,  # [K, B_SHARD] uint8, k-major input (X.T byte codes)
    g_d,  # [P, DCH * 2 * GW] uint8 packed G
    repeats,
    x_bufs=4,
    pp_bufs=4,
):
    nc = tc.nc
    x_f8 = x_d.bitcast(FP8)
    o_f8 = o_d.bitcast(FP8)
    g_f8 = g_d.bitcast(FP8)

    with contextlib.ExitStack() as stk:
        consts = stk.enter_context(tc.tile_pool(name="consts", bufs=1))
        xpool = stk.enter_context(tc.tile_pool(name="x", bufs=x_bufs))
        pppool = stk.enter_context(tc.tile_pool(name="pp", bufs=pp_bufs, space="PSUM"))
        paripool = stk.enter_context(tc.tile_pool(name="pari", bufs=2))
        parspool = stk.enter_context(tc.tile_pool(name="pars", bufs=2))

        # G host-packed as [128, DCH, 2, 32]: g[p, d, i, m] = G[256d+128i+p, m]
        # (cols 24:32 zero) so each matmul's stationary lhsT is a contiguous
        # DoubleRow slice with a 32B pair stride.
        g_sb = consts.tile([P, DCH, 2, GW], FP8)
        # G rides the store (scalar) ring, which is idle at pass start — on
        # the sync ring it would delay the first X load behind it in FIFO.
        nc.scalar.dma_start(
            out=g_sb,
            in_=g_f8.rearrange("p (d i m) -> p d i m", i=2, m=GW),
        )

        def one_pass():
            # parity for the whole pass, k-major [24, 2048]; stored once
            pars = parspool.tile([CRC, B_SHARD], FP8, tag="pars")
            for b in range(N_RB):
                cols = slice(b * RB, (b + 1) * RB)
                # [128 k, 32 chunks, RB rows] fp8 — X.T block rides through
                # SBUF once: matmul rhs and store source alike.
                x2 = xpool.tile([P, N_CHUNKS, RB], FP8, tag="x2")
                nc.sync.dma_start(
                    out=x2,
                    in_=x_f8[:, cols].rearrange("(c p) r -> p c r", p=P),
                )
                # ppT = sum_d G_d.T @ XT_d = (X @ G).T (rows 24:32 zero);
                # one accumulation group per 512-row sub-slice / PSUM bank
                ppT = pppool.tile([GW, RB], FP32)
                for d in range(DCH):
                    for s in range(N_MB):
                        nc.tensor.matmul(
                            ppT[:, s * MB : (s + 1) * MB],
                            g_sb[:, d],
                            x2[:, 2 * d : 2 * d + 2, s * MB : (s + 1) * MB],
                            start=(d == 0),
                            stop=(d == DCH - 1),
                            perf_mode=mybir.MatmulPerfMode.DoubleRow,
                        )
                # mod-2 of exact-integer f32 sums: cast i32, AND 1, to fp8
                pari = paripool.tile([CRC, RB], I32)
                nc.vector.tensor_copy(pari, ppT[0:CRC])
                nc.vector.tensor_scalar(
                    pari, pari, 1, None, mybir.AluOpType.bitwise_and
                )
                nc.vector.tensor_copy(pars[:, cols], pari)
                nc.scalar.dma_start(
                    out=o_f8[0:K, cols].rearrange("(c p) r -> p c r", p=P),
                    in_=x2,
                )
            nc.scalar.dma_start(out=o_f8[K : K + CRC, :], in_=pars)

        if repeats == 1:
            one_pass()
        else:
            with tc.For_i(0, repeats, 1):
                one_pass()


def pack_g(g_mat: np.ndarray) -> np.ndarray:
    """[4096, 24] {0,1} -> fp8-coded uint8 [128, DCH*2*32] with
    g[p, (d, i), 0:24] = G[256d+128i+p, :] and cols 24:32 zero."""
    g = np.asarray(g_mat)
    gp = np.zeros((P, DCH, 2, GW), np.uint8)
    gp[:, :, :, :CRC] = (
        g.reshape(DCH, 2, P, CRC).transpose(2, 0, 1, 3) != 0
    ).astype(np.uint8) * FP8_ONE
    return np.ascontiguousarray(gp.reshape(P, DCH * 2 * GW))


def encode_xt(x_shard: np.ndarray) -> np.ndarray:
    """{0,1} float32 [rows, K] -> k-major fp8 byte codes [K, rows] uint8."""
    return np.ascontiguousarray(x_shard.astype(np.uint8).T * FP8_ONE)


def decode_out_t(out_u8: np.ndarray) -> np.ndarray:
    """k-major fp8 byte codes [K+CRC, rows] back to {0,1} f32 [rows, K+CRC]."""
    return (out_u8.T != 0).astype(np.float32)


def build_crc_module(repeats: int = 1):
    nc = bacc.Bacc(
        "TRN2", target_bir_lowering=False, debug=False, num_devices=N_CORES
    )
    x_d = nc.dram_tensor("inputs_t", [K, B_SHARD], U8, kind="ExternalInput").ap()
    g_d = nc.dram_tensor(
        "g_packed", [P, DCH * 2 * GW], U8, kind="ExternalInput"
    ).ap()
    o_d = nc.dram_tensor(
        "out_t", [K + CRC, B_SHARD], U8, kind="ExternalOutput"
    ).ap()
    with TileContext(nc) as tc:
        _crc_body(tc, o_d, x_d, g_d, repeats)
    nc.compile()
    return nc


_NC_CACHE = None


def kernel(inputs: np.ndarray, g_mat: np.ndarray) -> np.ndarray:
    global _NC_CACHE
    if _NC_CACHE is None:
        _NC_CACHE = build_crc_module(repeats=1)
    nc = _NC_CACHE

    x = np.asarray(inputs, dtype=np.float32)
    g = np.asarray(g_mat, dtype=np.float32)
    assert x.shape == (BATCH, K) and g.shape == (K, CRC)
    gp = pack_g(g)

    in_maps = [
        {
            "inputs_t": encode_xt(x[i * B_SHARD : (i + 1) * B_SHARD]),
            "g_packed": gp,
        }
        for i in range(N_CORES)
    ]
    res = run_bass_kernel_spmd(nc, in_maps, core_ids=list(range(N_CORES)))
    out = np.concatenate(
        [decode_out_t(r["out_t"]) for r in res.results], axis=0
    )
    return out
